# revision 24
# baseline (speedup 1.0000x reference)
"""Trainium2 Bass kernel for nn_LongformerICD (B=2,S=2048,D=768,H=12,W=256,L=2).

Sharding: 8 cores = 2 batches x 4 sequence chunks of 512 tokens.
Each core processes an extended 1025-token window (512 own + 256 halo each
side + CLS slot) with halo recompute for K/V. Layer 2 is pruned to the
global-CLS-attention path (the classifier only reads position 0). Two tiny
AllReduces ([65,12] f32) combine global-attention partials within each
4-core batch group. Matmuls run in bf16 with f32 accumulation; residuals,
LayerNorm and softmax math stay f32.

Self-contained: hardcodes all shapes; no sibling imports.
"""
import numpy as np
import ml_dtypes

B, S, D, H, DH = 2, 2048, 768, 12, 64
W, FF, NL = 256, 3072, 50
VOCAB = 50265
EXT = 1025          # 8*128 halo-extended window + 1 CLS slot
OWN = 512
DT = D // 128       # 6 d-tiles
FT = FF // 128      # 24 ff-tiles
NEG = -240.0        # additive mask pre-softmax-scale: exp(-240*0.125)=exp(-30)
SCALE = 0.125       # 1/sqrt(64)
EPS = 1e-5

_CACHE = {}


# ----------------------------------------------------------------- host prep

def _geometry(j):
    """ext global indices [1025] + validity for chunk j."""
    g = 512 * j - 256 + np.arange(1024)
    valid = (g >= 0) & (g < S)
    gc = np.clip(g, 0, S - 1)
    gc[~valid] = 0
    ext_idx = np.concatenate([gc, [0]])
    ext_valid = np.concatenate([valid, [True]])
    return ext_idx, ext_valid


def _window_masks(j):
    """additive masks [2, 768, 256] f32 for scoresT (keys, queries)."""
    _, ext_valid = _geometry(j)
    masks = np.zeros((2, 768, 256), np.float32)
    for cc in range(2):
        k_local = 256 * cc + np.arange(768)
        k_g = 512 * j - 256 + k_local
        q_g = 512 * j + 256 * cc + np.arange(256)
        ok = (
            ext_valid[k_local][:, None]
            & (k_g[:, None] >= 1)
            & (k_g[:, None] < S)
            & (np.abs(k_g[:, None] - q_g[None, :]) <= W)
        )
        masks[cc] = np.where(ok, 0.0, NEG)
    return masks


def _bf(x):
    return np.ascontiguousarray(np.asarray(x, np.float32).astype(ml_dtypes.bfloat16))


def _f32(x):
    return np.ascontiguousarray(np.asarray(x, np.float32))


# ------------------------------------------------------------- device program

def _build_program():
    import concourse.bass as bass
    from concourse import bacc
    import concourse.tile as tile
    import concourse.mybir as mybir
    from concourse.masks import make_identity
    from contextlib import ExitStack

    f32 = mybir.dt.float32
    bf16 = mybir.dt.bfloat16
    i32 = mybir.dt.int32
    Alu = mybir.AluOpType
    Act = mybir.ActivationFunctionType

    nc = bacc.Bacc("TRN2", target_bir_lowering=False, debug=False, num_devices=8)

    # ---- I/O declarations
    ids = nc.dram_tensor("ids", [1026, 1], i32, kind="ExternalInput")
    pos_ext = nc.dram_tensor("pos_ext", [EXT, D], f32, kind="ExternalInput")
    masks_d = nc.dram_tensor("masks", [2, 768, 256], bf16, kind="ExternalInput")
    is_owner = nc.dram_tensor("is_owner", [128, 1], f32, kind="ExternalInput")
    tok_emb = nc.dram_tensor("tok_emb", [VOCAB, D], f32, kind="ExternalInput")
    lne_g = nc.dram_tensor("lne_g", [D], f32, kind="ExternalInput")
    lne_b = nc.dram_tensor("lne_b", [D], f32, kind="ExternalInput")

    wname = {}
    for i in range(2):
        names = ["wqg", "wkg", "wvg", "wo", "w1", "w2"] + (["wq", "wk", "wv"] if i == 0 else [])
        for n in names:
            shp = [D, FF] if n == "w1" else ([FF, D] if n == "w2" else [D, D])
            wname[f"{n}{i}"] = nc.dram_tensor(f"{n}{i}", shp, bf16, kind="ExternalInput")
        bnames = ["bqg", "bkg", "bvg", "bo", "b1", "b2", "ln1_g", "ln1_b", "ln2_g", "ln2_b"] + (
            ["bq", "bk", "bv"] if i == 0 else [])
        for n in bnames:
            shp = [FF] if n == "b1" else [D]
            wname[f"{n}{i}"] = nc.dram_tensor(f"{n}{i}", shp, f32, kind="ExternalInput")
    wc = nc.dram_tensor("wc", [D, NL], f32, kind="ExternalInput")
    bc = nc.dram_tensor("bc", [NL], f32, kind="ExternalInput")

    logits_o = nc.dram_tensor("logits", [1, NL], f32, kind="ExternalOutput")
    dbg_o = nc.dram_tensor("dbg", [128, 48], f32, kind="ExternalOutput")

    RG = [[0, 1, 2, 3], [4, 5, 6, 7]]

    with tile.TileContext(nc) as tc, ExitStack() as ctx:
        persist = ctx.enter_context(tc.tile_pool(name="persist", bufs=1))
        dram = ctx.enter_context(tc.tile_pool(name="dram", bufs=1, space="DRAM"))

        # ---- constants
        identity = persist.tile([128, 128], f32, tag="identity")
        make_identity(nc, identity[:])
        ones_bf = persist.tile([128, 1], bf16, tag="ones_bf")
        nc.vector.memset(ones_bf[:], 1.0)
        eps1 = persist.tile([1, 1], f32, tag="eps1")
        nc.vector.memset(eps1[:], EPS)
        eps128 = persist.tile([128, 1], f32, tag="eps128")
        nc.vector.memset(eps128[:], EPS)
        own_sb = persist.tile([128, 1], f32, tag="own_sb")
        nc.sync.dma_start(own_sb[:], is_owner[:])

        dbg_sb = persist.tile([128, 48], f32, tag="dbg")
        nc.vector.memset(dbg_sb[:], 0.0)

        # ---- lifetime-scoped activation pools (manual LIFO stack)
        cm_xa = tc.tile_pool(name="P_xa", bufs=1)
        pxa = cm_xa.__enter__()
        xa_f32 = pxa.tile([128, DT, 513], f32, tag="xa_f32")
        xa_bf = pxa.tile([128, DT, 513], bf16, tag="xa_bf")
        x1T_bf = pxa.tile([128, DT, 513], bf16, tag="x1T_bf")
        x1cls_col = pxa.tile([128, DT], f32, tag="x1cls_col")
        cm_x0 = tc.tile_pool(name="P_x0", bufs=1)
        px0 = cm_x0.__enter__()
        x0T_bf = px0.tile([128, DT, EXT], bf16, tag="x0T_bf")
        x0T_f32 = px0.tile([128, DT, 513], f32, tag="x0T_f32")  # own + CLS

        def bias_col(name, tiles=DT):
            """load f32 bias [tiles*128] as per-partition cols [128, tiles]"""
            t = persist.tile([128, tiles], f32, tag=f"bias_{name}")
            nc.sync.dma_start(t[:], wname[name][:].rearrange("(k p) -> p k", p=128))
            return t

        def bias_bcast(name, pool=None):
            """f32 [768] -> [128, 768] partition-broadcast"""
            t = (pool or persist).tile([128, D], f32, tag=f"bb_{name}")
            nc.sync.dma_start(t[:], wname[name][:][None].to_broadcast([128, D]))
            return t

        def load_w(pool, name, kt, n):
            t = pool.tile([128, kt, n], bf16, tag=name)
            nc.sync.dma_start(t[:], wname[name][:].rearrange("(k p) n -> p k n", p=128))
            return t

        # =========================================================== embedding
        with (
            tc.tile_pool(name="emb_sb", bufs=3) as esb,
            tc.tile_pool(name="emb_ps", bufs=3, space="PSUM") as eps_ps,
        ):
            g_bc = esb.tile([128, D], f32, tag="lne_g_bc")
            nc.sync.dma_start(g_bc[:], lne_g[:][None].to_broadcast([128, D]))
            b_bc = esb.tile([128, D], f32, tag="lne_b_bc")
            nc.sync.dma_start(b_bc[:], lne_b[:][None].to_broadcast([128, D]))

            for tb in range(9):
                pg = 128 if tb < 8 else 2   # gather rows (indirect DMA needs >=2)
                p = 128 if tb < 8 else 1    # rows actually used
                idx_sb = esb.tile([128, 1], i32, tag="idx")
                nc.sync.dma_start(idx_sb[:pg], ids[tb * 128: tb * 128 + pg])
                rows = esb.tile([128, D], f32, tag="rows")
                nc.gpsimd.indirect_dma_start(
                    out=rows[:pg], out_offset=None, in_=tok_emb[:],
                    in_offset=bass.IndirectOffsetOnAxis(ap=idx_sb[:pg, :1], axis=0))
                pos_sb = esb.tile([128, D], f32, tag="pos")
                nc.sync.dma_start(pos_sb[:p], pos_ext[tb * 128: tb * 128 + p])
                nc.vector.tensor_tensor(rows[:p], rows[:p], pos_sb[:p], op=Alu.add)
                # LayerNorm (natural layout, per-token stats)
                stats = esb.tile([128, 3, 6], f32, tag="bnst")
                for sg in range(3):
                    nc.vector.bn_stats(stats[:p, sg], rows[:p, sg * 256:(sg + 1) * 256])
                mv = esb.tile([128, 2], f32, tag="bnmv")
                nc.vector.bn_aggr(mv[:p], stats[:p])
                sd = esb.tile([128, 1], f32, tag="sd")
                nc.scalar.activation(sd[:p], mv[:p, 1:2], Act.Sqrt, bias=eps128[:p])
                nc.vector.reciprocal(sd[:p], sd[:p])
                xn = esb.tile([128, D], f32, tag="xn")
                nc.vector.tensor_scalar(xn[:p], rows[:p], mv[:p, 0:1], sd[:p, 0:1],
                                        Alu.subtract, Alu.mult)
                nc.vector.tensor_tensor(xn[:p], xn[:p], g_bc[:p], op=Alu.mult)
                nc.vector.tensor_tensor(xn[:p], xn[:p], b_bc[:p], op=Alu.add)
                # transpose to x0T
                pt = 1 if tb == 8 else 128
                for dt in range(DT):
                    tp = eps_ps.tile([128, 128], f32, tag="tp", space="PSUM")
                    nc.tensor.transpose(tp[:, :pt], xn[:pt, dt * 128:(dt + 1) * 128],
                                        identity[:pt, :pt])
                    nc.vector.tensor_copy(x0T_bf[:, dt, tb * 128: tb * 128 + pt],
                                          tp[:, :pt])
                    if 2 <= tb <= 5:
                        nc.scalar.copy(x0T_f32[:, dt, (tb - 2) * 128:(tb - 1) * 128],
                                       tp[:, :128])
                    if tb == 8:
                        nc.scalar.copy(x0T_f32[:, dt, 512:513], tp[:, :1])

        nc.vector.tensor_copy(dbg_sb[:, 0:6], x0T_f32[:, :, 0])

        # ================================================= shared helper defs
        def xT_proj(ps, sb, wt, src_bf, col_lo, col_hi, bcol, out_bf, out_name):
            """out[128, DT, n] = W^T @ src[:, :, lo:hi] + bias-cols (xT layout)."""
            n = col_hi - col_lo
            for ot in range(DT):
                slices = [(0, min(512, n))] + ([(512, n)] if n > 512 else [])
                for s0, s1 in slices:
                    acc = ps.tile([128, 512], f32, tag=f"{out_name}_ps", space="PSUM", bufs=2)
                    for kt in range(DT):
                        nc.tensor.matmul(
                            acc[:, : s1 - s0],
                            wt[:, kt, ot * 128:(ot + 1) * 128],
                            src_bf[:, kt, col_lo + s0: col_lo + s1],
                            start=(kt == 0), stop=(kt == DT - 1))
                    nc.vector.tensor_scalar(
                        out_bf[:, ot, s0:s1], acc[:, : s1 - s0],
                        bcol[:, ot: ot + 1], None, Alu.add)

        def nat_proj(ps, sb, wt, src_bf, blocks, bvec_bc, out_bf, out_name, pp=128):
            """natural-layout V = x @ Wv: out[128, nb, 768]; blocks = ext col starts."""
            for bi, c0 in enumerate(blocks):
                acc = ps.tile([128, D], f32, tag=f"{out_name}_ps", space="PSUM", bufs=2)
                for s0, s1 in ((0, 512), (512, 768)):
                    for kt in range(DT):
                        nc.tensor.matmul(
                            acc[:pp, s0:s1],
                            src_bf[:, kt, c0: c0 + pp],
                            wt[:, kt, s0:s1],
                            start=(kt == 0), stop=(kt == DT - 1))
                nc.vector.tensor_tensor(out_bf[:pp, bi], acc[:pp], bvec_bc[:pp],
                                        op=Alu.add)

        def row_proj(ps, wt, src_col_bf, kt_n, n_out, out_name):
            """row layout: out[1, n_out] = src[din]^T W ; src_col_bf [128, kt_n]."""
            acc = ps.tile([1, n_out], f32, tag=f"{out_name}_ps", space="PSUM", bufs=1)
            nsl = [(i * 512, min((i + 1) * 512, n_out)) for i in range((n_out + 511) // 512)]
            for s0, s1 in nsl:
                for kt in range(kt_n):
                    nc.tensor.matmul(acc[:, s0:s1], src_col_bf[:, kt: kt + 1],
                                     wt[:, kt, s0:s1],
                                     start=(kt == 0), stop=(kt == kt_n - 1))
            return acc

        def col_bounce(row_ap, n, tag, dtype=f32):
            """[1, n*128-elem row] -> col [128, n] via dram bounce. Returns tile."""
            d = dram.tile([n * 128], f32, tag=f"{tag}_d")
            nc.sync.dma_start(d[:][None], row_ap)
            t = persist.tile([128, n], dtype, tag=f"{tag}_c")
            eng = nc.gpsimd if dtype != f32 else nc.sync
            eng.dma_start(t[:], d[:].rearrange("(k p) -> p k", p=128))
            return t

        def ln_xT(ps, sb, sum_f32, n_cols, g_col, b_col, out_bf, out_f32, tag):
            """LayerNorm over partitions (d) in xT layout for [128, DT, n_cols]."""
            sum_bf = sb.tile([128, DT, n_cols], bf16, tag=f"{tag}_sbf", bufs=1)
            sq_bf = sb.tile([128, DT, n_cols], bf16, tag=f"{tag}_qbf", bufs=1)
            for dt in range(DT):
                nc.vector.tensor_copy(sum_bf[:, dt], sum_f32[:, dt])
                nc.vector.tensor_tensor(sq_bf[:, dt], sum_bf[:, dt], sum_bf[:, dt],
                                        op=Alu.mult)
            sx = ps.tile([1, n_cols], f32, tag=f"{tag}_sx", space="PSUM", bufs=1)
            sq = ps.tile([1, n_cols], f32, tag=f"{tag}_sq", space="PSUM", bufs=1)
            for (acc, src) in ((sx, sum_bf), (sq, sq_bf)):
                slices = [(0, min(512, n_cols))] + ([(512, n_cols)] if n_cols > 512 else [])
                for s0, s1 in slices:
                    for dt in range(DT):
                        nc.tensor.matmul(acc[:, s0:s1], ones_bf[:],
                                         src[:, dt, s0:s1],
                                         start=(dt == 0), stop=(dt == DT - 1))
            st = sb.tile([1, 2, n_cols], f32, tag=f"{tag}_st", bufs=1)
            nc.vector.tensor_scalar(st[:, 0], sx[:], 1.0 / D, None, Alu.mult)
            nc.vector.tensor_scalar(st[:, 1], sq[:], 1.0 / D, None, Alu.mult)
            m2 = sb.tile([1, n_cols], f32, tag=f"{tag}_m2", bufs=1)
            nc.vector.tensor_tensor(m2[:], st[:, 0], st[:, 0], op=Alu.mult)
            nc.vector.tensor_tensor(st[:, 1], st[:, 1], m2[:], op=Alu.subtract)
            nc.scalar.activation(st[:, 1], st[:, 1], Act.Sqrt, bias=eps1[:])
            nc.vector.reciprocal(st[:, 1], st[:, 1])
            d = dram.tile([2 * n_cols], f32, tag=f"{tag}_d")
            nc.sync.dma_start(d[:].rearrange("(a f) -> a f", a=2)[None], st[:, :, :])
            bcst = sb.tile([128, 2, n_cols], f32, tag=f"{tag}_bc", bufs=1)
            nc.sync.dma_start(
                bcst[:], d[:].rearrange("(a f) -> a f", a=2)[None].to_broadcast(
                    [128, 2, n_cols]))
            for dt in range(DT):
                t1 = sb.tile([128, n_cols], f32, tag=f"{tag}_t1")
                nc.vector.tensor_tensor(t1[:], sum_f32[:, dt], bcst[:, 0], op=Alu.subtract)
                nc.vector.tensor_tensor(t1[:], t1[:], bcst[:, 1], op=Alu.mult)
                if out_f32 is not None:
                    nc.vector.tensor_scalar(out_f32[:, dt], t1[:], g_col[:, dt:dt + 1],
                                            b_col[:, dt:dt + 1], Alu.mult, Alu.add)
                    nc.vector.tensor_copy(out_bf[:, dt], out_f32[:, dt])
                else:
                    nc.vector.tensor_scalar(out_bf[:, dt], t1[:], g_col[:, dt:dt + 1],
                                            b_col[:, dt:dt + 1], Alu.mult, Alu.add)
            return bcst

        # ==================================== layer-1 global partials + AR #1
        ar1_in = dram.tile([65, 12], f32, tag="ar1_in")
        ar1_out = dram.tile([65, 12], f32, tag="ar1_out")
        with (
            tc.tile_pool(name="g1_sb", bufs=2) as gsb,
            tc.tile_pool(name="g1_w", bufs=1) as gw,
        ):
            wqgt = load_w(gw, "wqg0", DT, D)
            wkgt = load_w(gw, "wkg0", DT, D)
            wvgt = load_w(gw, "wvg0", DT, D)
            bkg_col = bias_col("bkg0")
            bvg_bc = bias_bcast("bvg0", gsb)

            vg_bf = gsb.tile([128, 4, D], bf16, tag="vg1")
            kgT_bf = gsb.tile([128, DT, 512], bf16, tag="kg1")
            with tc.tile_pool(name="g1_ps", bufs=1, space="PSUM") as gps:
                # qg row = x0[CLS]^T Wqg + bqg
                qg_ps = row_proj(gps, wqgt, x0T_bf[:, :, 1024:1025].rearrange(
                    "p k one -> p (k one)"), DT, D, "qg1")
                qg_row = gsb.tile([1, D], f32, tag="qg1_row")
                bqg_row = gsb.tile([1, D], f32, tag="bqg_row")
                nc.sync.dma_start(bqg_row[:], wname["bqg0"][:][None])
                nc.vector.tensor_tensor(qg_row[:], qg_ps[:], bqg_row[:], op=Alu.add)
                qg_col = col_bounce(qg_row[:], DT, "qg1", bf16)

                nat_proj(gps, gsb, wvgt, x0T_bf, [256 + 128 * t for t in range(4)],
                         bvg_bc, vg_bf, "vg1")
                xT_proj(gps, gsb, wkgt, x0T_bf, 256, 768, bkg_col, kgT_bf, "kg1")

            part_sb = gsb.tile([65, 12], f32, tag="part1")
            expg = gsb.tile([128, 12, 4], bf16, tag="expg1")
            with tc.tile_pool(name="g1_ps2", bufs=1, space="PSUM") as gps2:
                for h in range(H):
                    po, pk = (h % 2) * 64, h // 2
                    for t in range(4):
                        sg = gps2.tile([128, 1], f32, tag="sg1", space="PSUM", bufs=2)
                        nc.tensor.matmul(sg[:, :], kgT_bf[po:po + 64, pk, 128 * t:128 * (t + 1)],
                                         qg_col[po:po + 64, pk:pk + 1], start=True, stop=True)
                        nc.scalar.activation(expg[:, h, t:t + 1], sg[:, :], Act.Exp,
                                             scale=SCALE)
                    num = gps2.tile([64, 1], f32, tag="num1", space="PSUM", bufs=2)
                    for t in range(4):
                        nc.tensor.matmul(num[:, :], vg_bf[:, t, h * 64:(h + 1) * 64],
                                         expg[:, h, t:t + 1], start=(t == 0), stop=(t == 3))
                    nc.vector.tensor_copy(part_sb[0:64, h:h + 1], num[:, :])
                    den = gps2.tile([1, 4], f32, tag="den1", space="PSUM", bufs=2)
                    nc.tensor.matmul(den[:, :], ones_bf[:], expg[:, h, 0:4],
                                     start=True, stop=True)
                    nc.vector.reduce_sum(part_sb[64:65, h:h + 1], den[:, :],
                                         axis=mybir.AxisListType.X)
            nc.sync.dma_start(ar1_in[:], part_sb[:])
            nc.gpsimd.collective_compute(
                "AllReduce", Alu.add, replica_groups=RG,
                ins=[ar1_in.opt()], outs=[ar1_out.opt()])

        # og1 columns [128, DT]: num interleaved + den broadcast, then divide
        og1_col = persist.tile([128, DT], f32, tag="og1_col")
        og1_den = persist.tile([128, DT], f32, tag="og1_den")
        for h2 in range(2):
            nc.sync.dma_start(
                og1_col[h2 * 64:(h2 + 1) * 64, :],
                ar1_out[0:64].rearrange("p (k h) -> h p k", h=2)[h2])
            nc.sync.dma_start(
                og1_den[h2 * 64:(h2 + 1) * 64, :],
                ar1_out[64:65].rearrange("one (k h) -> h one k", h=2)[h2].to_broadcast(
                    [64, DT]))
        nc.vector.reciprocal(og1_den[:], og1_den[:])
        nc.vector.tensor_tensor(og1_col[:], og1_col[:], og1_den[:], op=Alu.mult)
        nc.vector.tensor_copy(dbg_sb[:, 12:18], og1_col[:])

        # ============================================= layer-1 Q/K/V + window
        cm_qkv = tc.tile_pool(name="P_qkv", bufs=1)
        pqkv = cm_qkv.__enter__()
        QT_bf = pqkv.tile([128, DT, 512], bf16, tag="QT_bf")
        KT_bf = pqkv.tile([128, DT, EXT], bf16, tag="KT_bf")
        V_bf = pqkv.tile([128, 8, D], bf16, tag="V_bf")
        vcls_bf = pqkv.tile([1, D], bf16, tag="vcls_bf")
        with (
            tc.tile_pool(name="qkv_sb", bufs=2) as qsb,
            tc.tile_pool(name="qkv_w", bufs=1) as qw,
        ):
            wqt = load_w(qw, "wq0", DT, D)
            wkt = load_w(qw, "wk0", DT, D)
            wvt = load_w(qw, "wv0", DT, D)
            bq_col = bias_col("bq0")
            bk_col = bias_col("bk0")
            bv_bc = bias_bcast("bv0", qsb)
            with tc.tile_pool(name="qkv_ps", bufs=1, space="PSUM") as qps:
                xT_proj(qps, qsb, wqt, x0T_bf, 256, 768, bq_col, QT_bf, "QT")
                # KT over all 1025 ext cols
                for ot in range(DT):
                    for s0, s1 in ((0, 512), (512, 1024), (1024, 1025)):
                        acc = qps.tile([128, 512], f32, tag="KT_ps", space="PSUM", bufs=2)
                        for kt in range(DT):
                            nc.tensor.matmul(acc[:, : s1 - s0],
                                             wkt[:, kt, ot * 128:(ot + 1) * 128],
                                             x0T_bf[:, kt, s0:s1],
                                             start=(kt == 0), stop=(kt == DT - 1))
                        nc.vector.tensor_scalar(KT_bf[:, ot, s0:s1], acc[:, : s1 - s0],
                                                bk_col[:, ot:ot + 1], None, Alu.add)
                nat_proj(qps, qsb, wvt, x0T_bf, [128 * t for t in range(8)], bv_bc,
                         V_bf, "V")
            with tc.tile_pool(name="vcls_ps", bufs=1, space="PSUM") as vps:
                vc = row_proj(vps, wvt, x0T_bf[:, :, 1024:1025].rearrange(
                    "p k one -> p (k one)"), DT, D, "vcls")
                nc.vector.tensor_tensor(vcls_bf[:], vc[:], bv_bc[0:1], op=Alu.add)

        # window attention -> attnT numerators + L denominators
        cm_att = tc.tile_pool(name="P_att", bufs=1)
        patt = cm_att.__enter__()
        attnT = patt.tile([128, DT, 513], bf16, tag="attnT")
        Lden = patt.tile([1, 12, 513], f32, tag="Lden")
        masks_sb = patt.tile([128, 2, 6, 256], bf16, tag="masks_sb")
        nc.sync.dma_start(
            masks_sb[:], masks_d[:].rearrange("c (k p) q -> p c k q", p=128))
        with (
            tc.tile_pool(name="att_sb", bufs=3) as asb,
            tc.tile_pool(name="att_ps", bufs=2, space="PSUM") as aps,
        ):
            for cc in range(2):
                for h in range(H):
                    po, pk = (h % 2) * 64, h // 2
                    expT = asb.tile([128, 6, 256], bf16, tag="expT")
                    for kb in range(6):
                        sc = aps.tile([128, 256], f32, tag="sc", space="PSUM", bufs=2)
                        nc.tensor.matmul(
                            sc[:],
                            KT_bf[po:po + 64, pk, 256 * cc + 128 * kb: 256 * cc + 128 * (kb + 1)],
                            QT_bf[po:po + 64, pk, 256 * cc: 256 * (cc + 1)],
                            start=True, stop=True)
                        nc.vector.tensor_tensor(sc[:], sc[:], masks_sb[:, cc, kb],
                                                op=Alu.add)
                        nc.scalar.activation(expT[:, kb], sc[:], Act.Exp, scale=SCALE)
                    s0p = aps.tile([1, 256], f32, tag="s0", space="PSUM", bufs=2)
                    nc.tensor.matmul(s0p[:], KT_bf[po:po + 64, pk, 1024:1025],
                                     QT_bf[po:po + 64, pk, 256 * cc:256 * (cc + 1)],
                                     start=True, stop=True)
                    e0 = asb.tile([1, 256], bf16, tag="e0")
                    nc.scalar.activation(e0[:], s0p[:], Act.Exp, scale=SCALE)
                    # denominator
                    dn = aps.tile([1, 256], f32, tag="dn", space="PSUM", bufs=2)
                    for kb in range(6):
                        nc.tensor.matmul(dn[:], ones_bf[:], expT[:, kb],
                                         start=(kb == 0), stop=(kb == 5))
                    nc.vector.tensor_tensor(Lden[:, h, 256 * cc:256 * (cc + 1)],
                                            dn[:], e0[:], op=Alu.add)
                    # A @ V
                    av = aps.tile([64, 256], f32, tag="av", space="PSUM", bufs=2)
                    for kb in range(6):
                        nc.tensor.matmul(av[:], V_bf[:, 2 * cc + kb, h * 64:(h + 1) * 64],
                                         expT[:, kb], start=(kb == 0), stop=False)
                    nc.tensor.matmul(av[:], vcls_bf[:, h * 64:(h + 1) * 64], e0[:],
                                     start=False, stop=True)
                    nc.vector.tensor_copy(attnT[po:po + 64, pk, 256 * cc:256 * (cc + 1)],
                                          av[:])

        # normalize + og column + owner blend -> attn_norm_bf
        cm_nrm = tc.tile_pool(name="P_nrm", bufs=1)
        pnrm = cm_nrm.__enter__()
        attn_nbf = pnrm.tile([128, DT, 513], bf16, tag="attn_nbf")
        with tc.tile_pool(name="nrm_sb", bufs=2) as nsb:
            lr = nsb.tile([1, 12, 512], f32, tag="lr")
            nc.vector.reciprocal(lr[:], Lden[:, :, 0:512])
            lr_d = dram.tile([12, 512], f32, tag="lr_d")
            nc.sync.dma_start(lr_d[:][None], lr[:])
            lr_bc = nsb.tile([128, DT, 512], f32, tag="lr_bc")
            for h2 in range(2):
                nc.sync.dma_start(
                    lr_bc[h2 * 64:(h2 + 1) * 64],
                    lr_d[:].rearrange("(k h) f -> h k f", h=2)[h2][None].to_broadcast(
                        [64, DT, 512]))
            for dt in range(DT):
                nc.vector.tensor_tensor(attn_nbf[:, dt, 0:512], attnT[:, dt, 0:512],
                                        lr_bc[:, dt], op=Alu.mult)
                nc.vector.tensor_copy(attn_nbf[:, dt, 512:513], og1_col[:, dt:dt + 1])
            # owner blend on query col 0 (global position 0)
            d0 = nsb.tile([128, DT, 1], f32, tag="d0")
            nc.vector.tensor_tensor(d0[:], og1_col[:, :, None], attn_nbf[:, :, 0:1],
                                    op=Alu.subtract)
            nc.vector.tensor_tensor(d0[:], d0[:], own_sb[:, 0:1, None].to_broadcast(
                [128, DT, 1]), op=Alu.mult)
            nc.vector.tensor_tensor(attn_nbf[:, :, 0:1], attn_nbf[:, :, 0:1], d0[:],
                                    op=Alu.add)
        nc.vector.tensor_copy(dbg_sb[:, 6:12], attn_nbf[:, :, 0])

        # =============================================== O proj + LN1 + FFN
        with (
            tc.tile_pool(name="o_sb", bufs=2) as osb,
            tc.tile_pool(name="o_w", bufs=1) as ow,
            tc.tile_pool(name="o_ps", bufs=2, space="PSUM") as ops_,
        ):
            wot = load_w(ow, "wo0", DT, D)
            bo_col = bias_col("bo0")
            sum1 = osb.tile([128, DT, 513], f32, tag="sum1", bufs=1)
            for ot in range(DT):
                for s0, s1 in ((0, 512), (512, 513)):
                    acc = ops_.tile([128, 512], f32, tag="o_ps", space="PSUM", bufs=2)
                    for kt in range(DT):
                        nc.tensor.matmul(acc[:, : s1 - s0],
                                         wot[:, kt, ot * 128:(ot + 1) * 128],
                                         attn_nbf[:, kt, s0:s1],
                                         start=(kt == 0), stop=(kt == DT - 1))
                    t = osb.tile([128, 513], f32, tag="o_t")
                    nc.vector.tensor_scalar(t[:, s0:s1], acc[:, : s1 - s0],
                                            bo_col[:, ot:ot + 1], None, Alu.add)
                    nc.vector.tensor_tensor(sum1[:, ot, s0:s1], t[:, s0:s1],
                                            x0T_f32[:, ot, s0:s1], op=Alu.add)
            g1c = bias_col("ln1_g0")
            b1c = bias_col("ln1_b0")
            ln_xT(ops_, osb, sum1, 513, g1c, b1c, xa_bf, xa_f32, "ln1")
        nc.vector.tensor_copy(dbg_sb[:, 18:24], xa_f32[:, :, 0])

        # close attention-lifetime pools before the FFN phase (LIFO)
        cm_nrm.__exit__(None, None, None)
        cm_att.__exit__(None, None, None)
        cm_qkv.__exit__(None, None, None)
        cm_x0.__exit__(None, None, None)
        with (
            tc.tile_pool(name="f_sb", bufs=2) as fsb,
            tc.tile_pool(name="f_w", bufs=1) as fw,
            tc.tile_pool(name="f_ps", bufs=2, space="PSUM") as fps,
        ):
            w1t = load_w(fw, "w10", DT, FF)
            b1_col = bias_col("b10", FT)
            hT_bf = fsb.tile([128, FT, 513], bf16, tag="hT_bf", bufs=1)
            for ft in range(FT):
                for s0, s1 in ((0, 512), (512, 513)):
                    acc = fps.tile([128, 512], f32, tag="h_ps", space="PSUM", bufs=2)
                    for kt in range(DT):
                        nc.tensor.matmul(acc[:, : s1 - s0],
                                         w1t[:, kt, ft * 128:(ft + 1) * 128],
                                         xa_bf[:, kt, s0:s1],
                                         start=(kt == 0), stop=(kt == DT - 1))
                    nc.scalar.activation(hT_bf[:, ft, s0:s1], acc[:, : s1 - s0],
                                         Act.Gelu, bias=b1_col[:, ft:ft + 1])
            w2t = load_w(fw, "w20", FT, D)
            b2_col = bias_col("b20")
            sum2 = fsb.tile([128, DT, 513], f32, tag="sum2", bufs=1)
            for ot in range(DT):
                for s0, s1 in ((0, 512), (512, 513)):
                    acc = fps.tile([128, 512], f32, tag="f2_ps", space="PSUM", bufs=2)
                    for kt in range(FT):
                        nc.tensor.matmul(acc[:, : s1 - s0],
                                         w2t[:, kt, ot * 128:(ot + 1) * 128],
                                         hT_bf[:, kt, s0:s1],
                                         start=(kt == 0), stop=(kt == FT - 1))
                    t = fsb.tile([128, 513], f32, tag="f2_t")
                    nc.vector.tensor_scalar(t[:, s0:s1], acc[:, : s1 - s0],
                                            b2_col[:, ot:ot + 1], None, Alu.add)
                    nc.vector.tensor_tensor(sum2[:, ot, s0:s1], t[:, s0:s1],
                                            xa_f32[:, ot, s0:s1], op=Alu.add)
            g2c = bias_col("ln2_g0")
            b2c = bias_col("ln2_b0")
            st_bc = ln_xT(fps, fsb, sum2, 513, g2c, b2c, x1T_bf, None, "ln2")
            # f32 CLS column of x1 for the tail residual
            t1c = fsb.tile([128, DT, 1], f32, tag="t1c")
            nc.vector.tensor_tensor(t1c[:], sum2[:, :, 512:513],
                                    st_bc[:, 0:1, 512:513].to_broadcast([128, DT, 1]),
                                    op=Alu.subtract)
            nc.vector.tensor_tensor(t1c[:], t1c[:],
                                    st_bc[:, 1:2, 512:513].to_broadcast([128, DT, 1]),
                                    op=Alu.mult)
            for dt in range(DT):
                nc.vector.tensor_scalar(x1cls_col[:, dt:dt + 1], t1c[:, dt],
                                        g2c[:, dt:dt + 1], b2c[:, dt:dt + 1],
                                        Alu.mult, Alu.add)
        nc.vector.tensor_copy(dbg_sb[:, 24:30], x1cls_col[:])

        # ==================================== layer-2 global partials + AR #2
        ar2_in = dram.tile([65, 12], f32, tag="ar2_in")
        ar2_out = dram.tile([65, 12], f32, tag="ar2_out")
        with (
            tc.tile_pool(name="g2_sb", bufs=2) as gsb,
            tc.tile_pool(name="g2_w", bufs=1) as gw,
        ):
            wqgt = load_w(gw, "wqg1", DT, D)
            wkgt = load_w(gw, "wkg1", DT, D)
            wvgt = load_w(gw, "wvg1", DT, D)
            bkg_col = bias_col("bkg1")
            bvg_bc = bias_bcast("bvg1", gsb)
            vg_bf = gsb.tile([128, 4, D], bf16, tag="vg2")
            kgT_bf = gsb.tile([128, DT, 512], bf16, tag="kg2")
            with tc.tile_pool(name="g2_ps", bufs=1, space="PSUM") as gps:
                qg_ps = row_proj(gps, wqgt, x1T_bf[:, :, 512:513].rearrange(
                    "p k one -> p (k one)"), DT, D, "qg2")
                qg_row = gsb.tile([1, D], f32, tag="qg2_row")
                bqg_row = gsb.tile([1, D], f32, tag="bqg2_row")
                nc.sync.dma_start(bqg_row[:], wname["bqg1"][:][None])
                nc.vector.tensor_tensor(qg_row[:], qg_ps[:], bqg_row[:], op=Alu.add)
                qg_col = col_bounce(qg_row[:], DT, "qg2", bf16)

                nat_proj(gps, gsb, wvgt, x1T_bf, [128 * t for t in range(4)],
                         bvg_bc, vg_bf, "vg2")
                xT_proj(gps, gsb, wkgt, x1T_bf, 0, 512, bkg_col, kgT_bf, "kg2")

            part_sb = gsb.tile([65, 12], f32, tag="part2")
            expg = gsb.tile([128, 12, 4], bf16, tag="expg2")
            with tc.tile_pool(name="g2_ps2", bufs=1, space="PSUM") as gps2:
                for h in range(H):
                    po, pk = (h % 2) * 64, h // 2
                    for t in range(4):
                        sg = gps2.tile([128, 1], f32, tag="sg2", space="PSUM", bufs=2)
                        nc.tensor.matmul(sg[:, :], kgT_bf[po:po + 64, pk, 128 * t:128 * (t + 1)],
                                         qg_col[po:po + 64, pk:pk + 1], start=True, stop=True)
                        nc.scalar.activation(expg[:, h, t:t + 1], sg[:, :], Act.Exp,
                                             scale=SCALE)
                    num = gps2.tile([64, 1], f32, tag="num2", space="PSUM", bufs=2)
                    for t in range(4):
                        nc.tensor.matmul(num[:, :], vg_bf[:, t, h * 64:(h + 1) * 64],
                                         expg[:, h, t:t + 1], start=(t == 0), stop=(t == 3))
                    nc.vector.tensor_copy(part_sb[0:64, h:h + 1], num[:, :])
                    den = gps2.tile([1, 4], f32, tag="den2", space="PSUM", bufs=2)
                    nc.tensor.matmul(den[:, :], ones_bf[:], expg[:, h, 0:4],
                                     start=True, stop=True)
                    nc.vector.reduce_sum(part_sb[64:65, h:h + 1], den[:, :],
                                         axis=mybir.AxisListType.X)
            nc.sync.dma_start(ar2_in[:], part_sb[:])
            nc.gpsimd.collective_compute(
                "AllReduce", Alu.add, replica_groups=RG,
                ins=[ar2_in.opt()], outs=[ar2_out.opt()])

        # ============================================================== tail
        with (
            tc.tile_pool(name="t_sb", bufs=1) as tsb,
            tc.tile_pool(name="t_w", bufs=1) as tw,
        ):
            og2_col = tsb.tile([128, DT], f32, tag="og2_col")
            og2_den = tsb.tile([128, DT], f32, tag="og2_den")
            for h2 in range(2):
                nc.sync.dma_start(
                    og2_col[h2 * 64:(h2 + 1) * 64, :],
                    ar2_out[0:64].rearrange("p (k h) -> h p k", h=2)[h2])
                nc.sync.dma_start(
                    og2_den[h2 * 64:(h2 + 1) * 64, :],
                    ar2_out[64:65].rearrange("one (k h) -> h one k", h=2)[h2]
                    .to_broadcast([64, DT]))
            nc.vector.reciprocal(og2_den[:], og2_den[:])
            nc.vector.tensor_tensor(og2_col[:], og2_col[:], og2_den[:], op=Alu.mult)
            nc.vector.tensor_copy(dbg_sb[:, 30:36], og2_col[:])
            og2_bf = tsb.tile([128, DT], bf16, tag="og2_bf")
            nc.vector.tensor_copy(og2_bf[:], og2_col[:])

            wot = load_w(tw, "wo1", DT, D)

            # x1cls row via bounce
            x1d = dram.tile([D], f32, tag="x1cls_d")
            nc.sync.dma_start(x1d[:].rearrange("(k p) -> p k", p=128), x1cls_col[:])
            x1row = tsb.tile([1, D], f32, tag="x1row")
            nc.sync.dma_start(x1row[:], x1d[:][None])

            def row_ln(src_ps_row, resid_row, gname, bname_ln, proj_bias, tag):
                """x = LN(resid + src + proj_bias_row) in row layout [1, 768]."""
                brow = tsb.tile([1, D], f32, tag=f"{tag}_brow")
                nc.sync.dma_start(brow[:], wname[proj_bias][:][None])
                srow = tsb.tile([1, D], f32, tag=f"{tag}_srow")
                nc.vector.tensor_tensor(srow[:], src_ps_row[:], brow[:], op=Alu.add)
                nc.vector.tensor_tensor(srow[:], srow[:], resid_row[:], op=Alu.add)
                stats = tsb.tile([1, 3, 6], f32, tag=f"{tag}_bn")
                for sg3 in range(3):
                    nc.vector.bn_stats(stats[:, sg3], srow[:, sg3 * 256:(sg3 + 1) * 256])
                mv = tsb.tile([1, 2], f32, tag=f"{tag}_mv")
                nc.vector.bn_aggr(mv[:], stats[:])
                sd = tsb.tile([1, 1], f32, tag=f"{tag}_sd")
                nc.scalar.activation(sd[:], mv[:, 1:2], Act.Sqrt, bias=eps1[:])
                nc.vector.reciprocal(sd[:], sd[:])
                xr = tsb.tile([1, D], f32, tag=f"{tag}_x")
                nc.vector.tensor_scalar(xr[:], srow[:], mv[:, 0:1], sd[:, 0:1],
                                        Alu.subtract, Alu.mult)
                grow = tsb.tile([1, D], f32, tag=f"{tag}_grow")
                nc.sync.dma_start(grow[:], wname[gname][:][None])
                nc.vector.tensor_tensor(xr[:], xr[:], grow[:], op=Alu.mult)
                brow2 = tsb.tile([1, D], f32, tag=f"{tag}_brow2")
                nc.sync.dma_start(brow2[:], wname[bname_ln][:][None])
                nc.vector.tensor_tensor(xr[:], xr[:], brow2[:], op=Alu.add)
                return srow, xr

            with tc.tile_pool(name="tpsA", bufs=1, space="PSUM") as tpsA:
                a2 = row_proj(tpsA, wot, og2_bf, DT, D, "a2")
                _, x2a_row = row_ln(a2, x1row, "ln1_g1", "ln1_b1", "bo1", "tln1")
            x2a_col = col_bounce(x2a_row[:], DT, "x2a", bf16)

            with tc.tile_pool(name="t_w1", bufs=1) as tw1:
                w1t = load_w(tw1, "w11", DT, FF)
                with tc.tile_pool(name="tpsB", bufs=1, space="PSUM") as tpsB:
                    h2_ps = row_proj(tpsB, w1t, x2a_col, DT, FF, "h2")
                    b1row = tsb.tile([1, FF], f32, tag="b1row")
                    nc.sync.dma_start(b1row[:], wname["b11"][:][None])
                    h2row = tsb.tile([1, FF], f32, tag="h2row")
                    nc.vector.tensor_tensor(h2row[:], h2_ps[:], b1row[:], op=Alu.add)
                    nc.scalar.activation(h2row[:], h2row[:], Act.Gelu)
            h2_col = col_bounce(h2row[:], FT, "h2c", bf16)

            with tc.tile_pool(name="t_w2", bufs=1) as tw2:
                w2t = load_w(tw2, "w21", FT, D)
                with tc.tile_pool(name="tpsC", bufs=1, space="PSUM") as tpsC:
                    f2 = row_proj(tpsC, w2t, h2_col, FT, D, "f2")
                    _, x2_row = row_ln(f2, x2a_row, "ln2_g1", "ln2_b1", "b21", "tln2")

            x2_col = col_bounce(x2_row[:], DT, "x2c", f32)
            wc_sb = tsb.tile([128, DT, NL], f32, tag="wc_sb")
            nc.sync.dma_start(wc_sb[:], wc[:].rearrange("(k p) n -> p k n", p=128))
            with tc.tile_pool(name="tpsD", bufs=1, space="PSUM") as tpsD:
                lg = tpsD.tile([1, NL], f32, tag="lg_ps", space="PSUM")
                for kt in range(DT):
                    nc.tensor.matmul(lg[:], x2_col[:, kt:kt + 1], wc_sb[:, kt],
                                     start=(kt == 0), stop=(kt == DT - 1))
                bc_row = tsb.tile([1, NL], f32, tag="bc_row")
                nc.sync.dma_start(bc_row[:], bc[:][None])
                lrow = tsb.tile([1, NL], f32, tag="lrow")
                nc.vector.tensor_tensor(lrow[:], lg[:], bc_row[:], op=Alu.add)
            nc.sync.dma_start(logits_o[:], lrow[:])
            nc.sync.dma_start(dbg_o[:], dbg_sb[:])
        cm_xa.__exit__(None, None, None)

    nc.compile()
    return nc


# ------------------------------------------------------------------ host glue

def _prep_inputs(input_ids, attention_mask, params):
    p = {k: np.asarray(v) for k, v in params.items()}
    ids_np = np.asarray(input_ids).astype(np.int32)

    shared = {
        "tok_emb": _f32(p["tok_emb"]),
        "lne_g": _f32(p["ln_e_g"]), "lne_b": _f32(p["ln_e_b"]),
        "wc": _f32(p["Wc"]), "bc": _f32(p["bc"]),
    }
    for i in range(2):
        names = [("wqg", "Wqg"), ("wkg", "Wkg"), ("wvg", "Wvg"), ("wo", "Wo"),
                 ("w1", "W1"), ("w2", "W2")] + (
            [("wq", "Wq"), ("wk", "Wk"), ("wv", "Wv")] if i == 0 else [])
        for n, rn in names:
            shared[f"{n}{i}"] = _bf(p[rn][i])
        bn = [("bqg", "bqg"), ("bkg", "bkg"), ("bvg", "bvg"), ("bo", "bo"),
              ("b1", "b1"), ("b2", "b2"), ("ln1_g", "ln1_g"), ("ln1_b", "ln1_b"),
              ("ln2_g", "ln2_g"), ("ln2_b", "ln2_b")] + (
            [("bq", "bq"), ("bk", "bk"), ("bv", "bv")] if i == 0 else [])
        for n, rn in bn:
            shared[f"{n}{i}"] = _f32(p[rn][i])

    pos = _f32(p["pos_emb"])
    in_maps = []
    for c in range(8):
        b, j = c // 4, c % 4
        ext_idx, _ = _geometry(j)
        ids_ext = ids_np[b][ext_idx]
        ids_in = np.concatenate([ids_ext, ids_ext[-1:]]).reshape(1026, 1)
        m = dict(shared)
        m["ids"] = np.ascontiguousarray(ids_in)
        m["pos_ext"] = np.ascontiguousarray(pos[ext_idx])
        m["masks"] = _bf(_window_masks(j))
        m["is_owner"] = np.full((128, 1), 1.0 if j == 0 else 0.0, np.float32)
        in_maps.append(m)
    return in_maps


def kernel(input_ids, attention_mask, params):
    from concourse.bass_utils import run_bass_kernel_spmd

    if "nc" not in _CACHE:
        _CACHE["nc"] = _build_program()
    nc = _CACHE["nc"]
    in_maps = _prep_inputs(input_ids, attention_mask, params)
    r = run_bass_kernel_spmd(nc, in_maps, core_ids=list(range(8)))
    _CACHE["last_results"] = r
    out = np.stack([r.results[0]["logits"][0], r.results[4]["logits"][0]], 0)
    return out.astype(np.float32)


# revision 27
# speedup vs baseline: 1.0997x; 1.0997x over previous
"""Trainium2 Bass kernel for nn_LongformerICD (B=2,S=2048,D=768,H=12,W=256,L=2).

Sharding: 8 cores = 2 batches x 4 sequence chunks of 512 tokens.
Each core processes an extended 1025-token window (512 own + 256 halo each
side + CLS slot) with halo recompute for K/V. Layer 2 is pruned to the
global-CLS-attention path (the classifier only reads position 0). Two tiny
AllReduces ([65,12] f32) combine global-attention partials within each
4-core batch group. Matmuls run in bf16 with f32 accumulation; residuals,
LayerNorm and softmax math stay f32.

Self-contained: hardcodes all shapes; no sibling imports.
"""
import numpy as np
import ml_dtypes

B, S, D, H, DH = 2, 2048, 768, 12, 64
W, FF, NL = 256, 3072, 50
VOCAB = 50265
EXT = 1025          # 8*128 halo-extended window + 1 CLS slot
OWN = 512
DT = D // 128       # 6 d-tiles
FT = FF // 128      # 24 ff-tiles
NEG = -240.0        # additive mask pre-softmax-scale: exp(-240*0.125)=exp(-30)
SCALE = 0.125       # 1/sqrt(64)
EPS = 1e-5

_CACHE = {}


# ----------------------------------------------------------------- host prep

def _geometry(j):
    """ext global indices [1025] + validity for chunk j."""
    g = 512 * j - 256 + np.arange(1024)
    valid = (g >= 0) & (g < S)
    gc = np.clip(g, 0, S - 1)
    gc[~valid] = 0
    ext_idx = np.concatenate([gc, [0]])
    ext_valid = np.concatenate([valid, [True]])
    return ext_idx, ext_valid


def _range_bias(j):
    """post-scale additive key bias [2, 2, 128] for fully-in-band tiles kb=2,3."""
    _, ext_valid = _geometry(j)
    rb = np.zeros((2, 2, 128), np.float32)
    for cc in range(2):
        for kbi, kb in enumerate((2, 3)):
            k_local = 256 * cc + 128 * kb + np.arange(128)
            k_g = 512 * j - 256 + k_local
            ok = ext_valid[k_local] & (k_g >= 1) & (k_g < S)
            rb[cc, kbi] = np.where(ok, 0.0, -30.0)
    return rb


def _window_masks(j):
    """additive masks [2, 768, 256] f32 for scoresT (keys, queries)."""
    _, ext_valid = _geometry(j)
    masks = np.zeros((2, 768, 256), np.float32)
    for cc in range(2):
        k_local = 256 * cc + np.arange(768)
        k_g = 512 * j - 256 + k_local
        q_g = 512 * j + 256 * cc + np.arange(256)
        ok = (
            ext_valid[k_local][:, None]
            & (k_g[:, None] >= 1)
            & (k_g[:, None] < S)
            & (np.abs(k_g[:, None] - q_g[None, :]) <= W)
        )
        masks[cc] = np.where(ok, 0.0, NEG)
    return masks


def _bf(x):
    return np.ascontiguousarray(np.asarray(x, np.float32).astype(ml_dtypes.bfloat16))


def _f32(x):
    return np.ascontiguousarray(np.asarray(x, np.float32))


# ------------------------------------------------------------- device program

def _build_program():
    import concourse.bass as bass
    from concourse import bacc
    import concourse.tile as tile
    import concourse.mybir as mybir
    from concourse.masks import make_identity
    from contextlib import ExitStack

    f32 = mybir.dt.float32
    bf16 = mybir.dt.bfloat16
    i32 = mybir.dt.int32
    Alu = mybir.AluOpType
    Act = mybir.ActivationFunctionType

    nc = bacc.Bacc("TRN2", target_bir_lowering=False, debug=False, num_devices=8)

    # ---- I/O declarations
    ids = nc.dram_tensor("ids", [1026, 1], i32, kind="ExternalInput")
    pos_ext = nc.dram_tensor("pos_ext", [EXT, D], f32, kind="ExternalInput")
    masks_d = nc.dram_tensor("masks", [2, 768, 256], bf16, kind="ExternalInput")
    is_owner = nc.dram_tensor("is_owner", [128, 1], f32, kind="ExternalInput")
    range_d = nc.dram_tensor("range_bias", [2, 2, 128], f32, kind="ExternalInput")
    tok_emb = nc.dram_tensor("tok_emb", [VOCAB, D], f32, kind="ExternalInput")
    lne_g = nc.dram_tensor("lne_g", [D], f32, kind="ExternalInput")
    lne_b = nc.dram_tensor("lne_b", [D], f32, kind="ExternalInput")

    wname = {}
    for i in range(2):
        names = ["wqg", "wkg", "wvg", "wo", "w1", "w2"] + (["wq", "wk", "wv"] if i == 0 else [])
        for n in names:
            shp = [D, FF] if n == "w1" else ([FF, D] if n == "w2" else [D, D])
            wname[f"{n}{i}"] = nc.dram_tensor(f"{n}{i}", shp, bf16, kind="ExternalInput")
        bnames = ["bqg", "bkg", "bvg", "bo", "b1", "b2", "ln1_g", "ln1_b", "ln2_g", "ln2_b"] + (
            ["bq", "bk", "bv"] if i == 0 else [])
        for n in bnames:
            shp = [FF] if n == "b1" else [D]
            wname[f"{n}{i}"] = nc.dram_tensor(f"{n}{i}", shp, f32, kind="ExternalInput")
    wc = nc.dram_tensor("wc", [D, NL], f32, kind="ExternalInput")
    bc = nc.dram_tensor("bc", [NL], f32, kind="ExternalInput")

    logits_o = nc.dram_tensor("logits", [1, NL], f32, kind="ExternalOutput")
    dbg_o = nc.dram_tensor("dbg", [128, 48], f32, kind="ExternalOutput")

    RG = [[0, 1, 2, 3], [4, 5, 6, 7]]

    with tile.TileContext(nc) as tc, ExitStack() as ctx:
        persist = ctx.enter_context(tc.tile_pool(name="persist", bufs=1))
        dram = ctx.enter_context(tc.tile_pool(name="dram", bufs=1, space="DRAM"))

        # ---- constants
        identity = persist.tile([128, 128], f32, tag="identity")
        make_identity(nc, identity[:])
        ones_bf = persist.tile([128, 1], bf16, tag="ones_bf")
        nc.vector.memset(ones_bf[:], 1.0)
        eps1 = persist.tile([1, 1], f32, tag="eps1")
        nc.vector.memset(eps1[:], EPS)
        eps128 = persist.tile([128, 1], f32, tag="eps128")
        nc.vector.memset(eps128[:], EPS)
        own_sb = persist.tile([128, 1], f32, tag="own_sb")
        nc.sync.dma_start(own_sb[:], is_owner[:])

        dbg_sb = persist.tile([128, 48], f32, tag="dbg")
        nc.vector.memset(dbg_sb[:], 0.0)

        # ---- lifetime-scoped activation pools (manual LIFO stack)
        cm_xa = tc.tile_pool(name="P_xa", bufs=1)
        pxa = cm_xa.__enter__()
        xa_f32 = pxa.tile([128, DT, 513], f32, tag="xa_f32")
        xa_bf = pxa.tile([128, DT, 513], bf16, tag="xa_bf")
        x1T_bf = pxa.tile([128, DT, 513], bf16, tag="x1T_bf")
        x1cls_col = pxa.tile([128, DT], f32, tag="x1cls_col")
        cm_x0 = tc.tile_pool(name="P_x0", bufs=1)
        px0 = cm_x0.__enter__()
        x0T_bf = px0.tile([128, DT, EXT], bf16, tag="x0T_bf")
        x0T_f32 = px0.tile([128, DT, 513], f32, tag="x0T_f32")  # own + CLS

        def bias_col(name, tiles=DT):
            """load f32 bias [tiles*128] as per-partition cols [128, tiles]"""
            t = persist.tile([128, tiles], f32, tag=f"bias_{name}")
            nc.sync.dma_start(t[:], wname[name][:].rearrange("(k p) -> p k", p=128))
            return t

        def bias_bcast(name, pool=None):
            """f32 [768] -> [128, 768] partition-broadcast"""
            t = (pool or persist).tile([128, D], f32, tag=f"bb_{name}")
            nc.sync.dma_start(t[:], wname[name][:][None].to_broadcast([128, D]))
            return t

        def load_w(pool, name, kt, n):
            t = pool.tile([128, kt, n], bf16, tag=name)
            nc.sync.dma_start(t[:], wname[name][:].rearrange("(k p) n -> p k n", p=128))
            return t

        # =========================================================== embedding
        with (
            tc.tile_pool(name="emb_sb", bufs=3) as esb,
            tc.tile_pool(name="emb_ps", bufs=3, space="PSUM") as eps_ps,
        ):
            g_bc = esb.tile([128, D], f32, tag="lne_g_bc")
            nc.sync.dma_start(g_bc[:], lne_g[:][None].to_broadcast([128, D]))
            b_bc = esb.tile([128, D], f32, tag="lne_b_bc")
            nc.sync.dma_start(b_bc[:], lne_b[:][None].to_broadcast([128, D]))

            for tb in range(9):
                pg = 128 if tb < 8 else 2   # gather rows (indirect DMA needs >=2)
                p = 128 if tb < 8 else 1    # rows actually used
                idx_sb = esb.tile([128, 1], i32, tag="idx")
                nc.sync.dma_start(idx_sb[:pg], ids[tb * 128: tb * 128 + pg])
                rows = esb.tile([128, D], f32, tag="rows")
                nc.gpsimd.indirect_dma_start(
                    out=rows[:pg], out_offset=None, in_=tok_emb[:],
                    in_offset=bass.IndirectOffsetOnAxis(ap=idx_sb[:pg, :1], axis=0))
                pos_sb = esb.tile([128, D], f32, tag="pos")
                nc.sync.dma_start(pos_sb[:p], pos_ext[tb * 128: tb * 128 + p])
                nc.vector.tensor_tensor(rows[:p], rows[:p], pos_sb[:p], op=Alu.add)
                # LayerNorm (natural layout, per-token stats)
                stats = esb.tile([128, 3, 6], f32, tag="bnst")
                for sg in range(3):
                    nc.vector.bn_stats(stats[:p, sg], rows[:p, sg * 256:(sg + 1) * 256])
                mv = esb.tile([128, 2], f32, tag="bnmv")
                nc.vector.bn_aggr(mv[:p], stats[:p])
                sd = esb.tile([128, 1], f32, tag="sd")
                nc.scalar.activation(sd[:p], mv[:p, 1:2], Act.Sqrt, bias=eps128[:p])
                nc.vector.reciprocal(sd[:p], sd[:p])
                xn = esb.tile([128, D], f32, tag="xn")
                nc.vector.tensor_scalar(xn[:p], rows[:p], mv[:p, 0:1], sd[:p, 0:1],
                                        Alu.subtract, Alu.mult)
                nc.vector.tensor_tensor(xn[:p], xn[:p], g_bc[:p], op=Alu.mult)
                nc.vector.tensor_tensor(xn[:p], xn[:p], b_bc[:p], op=Alu.add)
                # transpose to x0T
                pt = 1 if tb == 8 else 128
                for dt in range(DT):
                    tp = eps_ps.tile([128, 128], f32, tag="tp", space="PSUM")
                    nc.tensor.transpose(tp[:, :pt], xn[:pt, dt * 128:(dt + 1) * 128],
                                        identity[:pt, :pt])
                    nc.vector.tensor_copy(x0T_bf[:, dt, tb * 128: tb * 128 + pt],
                                          tp[:, :pt])
                    if 2 <= tb <= 5:
                        nc.scalar.copy(x0T_f32[:, dt, (tb - 2) * 128:(tb - 1) * 128],
                                       tp[:, :128])
                    if tb == 8:
                        nc.scalar.copy(x0T_f32[:, dt, 512:513], tp[:, :1])

        nc.vector.tensor_copy(dbg_sb[:, 0:6], x0T_f32[:, :, 0])

        # ================================================= shared helper defs
        def xT_proj(ps, sb, wt, src_bf, col_lo, col_hi, bcol, out_bf, out_name):
            """out[128, DT, n] = W^T @ src[:, :, lo:hi] + bias-cols (xT layout)."""
            n = col_hi - col_lo
            for ot in range(DT):
                slices = [(0, min(512, n))] + ([(512, n)] if n > 512 else [])
                for s0, s1 in slices:
                    acc = ps.tile([128, 512], f32, tag=f"{out_name}_ps", space="PSUM", bufs=2)
                    for kt in range(DT):
                        nc.tensor.matmul(
                            acc[:, : s1 - s0],
                            wt[:, kt, ot * 128:(ot + 1) * 128],
                            src_bf[:, kt, col_lo + s0: col_lo + s1],
                            start=(kt == 0), stop=(kt == DT - 1))
                    nc.vector.tensor_scalar(
                        out_bf[:, ot, s0:s1], acc[:, : s1 - s0],
                        bcol[:, ot: ot + 1], None, Alu.add)

        def nat_proj(ps, sb, wt, src_bf, blocks, bvec_bc, out_bf, out_name, pp=128):
            """natural-layout V = x @ Wv: out[128, nb, 768]; blocks = ext col starts."""
            for bi, c0 in enumerate(blocks):
                acc = ps.tile([128, D], f32, tag=f"{out_name}_ps", space="PSUM", bufs=2)
                for s0, s1 in ((0, 512), (512, 768)):
                    for kt in range(DT):
                        nc.tensor.matmul(
                            acc[:pp, s0:s1],
                            src_bf[:, kt, c0: c0 + pp],
                            wt[:, kt, s0:s1],
                            start=(kt == 0), stop=(kt == DT - 1))
                nc.vector.tensor_tensor(out_bf[:pp, bi], acc[:pp], bvec_bc[:pp],
                                        op=Alu.add)

        def row_proj(ps, wt, src_col_bf, kt_n, n_out, out_name):
            """row layout: out[1, n_out] = src[din]^T W ; src_col_bf [128, kt_n]."""
            acc = ps.tile([1, n_out], f32, tag=f"{out_name}_ps", space="PSUM", bufs=1)
            nsl = [(i * 512, min((i + 1) * 512, n_out)) for i in range((n_out + 511) // 512)]
            for s0, s1 in nsl:
                for kt in range(kt_n):
                    nc.tensor.matmul(acc[:, s0:s1], src_col_bf[:, kt: kt + 1],
                                     wt[:, kt, s0:s1],
                                     start=(kt == 0), stop=(kt == kt_n - 1))
            return acc

        def col_bounce(row_ap, n, tag, dtype=f32):
            """[1, n*128-elem row] -> col [128, n] via dram bounce. Returns tile."""
            d = dram.tile([n * 128], f32, tag=f"{tag}_d")
            nc.sync.dma_start(d[:][None], row_ap)
            t = persist.tile([128, n], dtype, tag=f"{tag}_c")
            eng = nc.gpsimd if dtype != f32 else nc.sync
            eng.dma_start(t[:], d[:].rearrange("(k p) -> p k", p=128))
            return t

        def ln_xT(ps, sb, sum_f32, n_cols, g_col, b_col, out_bf, out_f32, tag):
            """LayerNorm over partitions (d) in xT layout for [128, DT, n_cols]."""
            sum_bf = sb.tile([128, DT, n_cols], bf16, tag=f"{tag}_sbf", bufs=1)
            sq_bf = sb.tile([128, DT, n_cols], bf16, tag=f"{tag}_qbf", bufs=1)
            for dt in range(DT):
                eng = nc.vector if dt % 2 == 0 else nc.gpsimd
                nc.scalar.copy(sum_bf[:, dt], sum_f32[:, dt])
                eng.tensor_tensor(sq_bf[:, dt], sum_bf[:, dt], sum_bf[:, dt],
                                  op=Alu.mult)
            sx = ps.tile([1, n_cols], f32, tag=f"{tag}_sx", space="PSUM", bufs=1)
            sq = ps.tile([1, n_cols], f32, tag=f"{tag}_sq", space="PSUM", bufs=1)
            for (acc, src) in ((sx, sum_bf), (sq, sq_bf)):
                slices = [(0, min(512, n_cols))] + ([(512, n_cols)] if n_cols > 512 else [])
                for s0, s1 in slices:
                    for dt in range(DT):
                        nc.tensor.matmul(acc[:, s0:s1], ones_bf[:],
                                         src[:, dt, s0:s1],
                                         start=(dt == 0), stop=(dt == DT - 1))
            st = sb.tile([1, 2, n_cols], f32, tag=f"{tag}_st", bufs=1)
            nc.vector.tensor_scalar(st[:, 0], sx[:], 1.0 / D, None, Alu.mult)
            nc.vector.tensor_scalar(st[:, 1], sq[:], 1.0 / D, None, Alu.mult)
            m2 = sb.tile([1, n_cols], f32, tag=f"{tag}_m2", bufs=1)
            nc.vector.tensor_tensor(m2[:], st[:, 0], st[:, 0], op=Alu.mult)
            nc.vector.tensor_tensor(st[:, 1], st[:, 1], m2[:], op=Alu.subtract)
            nc.scalar.activation(st[:, 1], st[:, 1], Act.Sqrt, bias=eps1[:])
            nc.vector.reciprocal_approx_fast(out=st[:, 1], in_=st[:, 1])
            d = dram.tile([2 * n_cols], f32, tag=f"{tag}_d")
            nc.sync.dma_start(d[:].rearrange("(a f) -> a f", a=2)[None], st[:, :, :])
            bcst = sb.tile([128, 2, n_cols], f32, tag=f"{tag}_bc", bufs=1)
            nc.sync.dma_start(
                bcst[:], d[:].rearrange("(a f) -> a f", a=2)[None].to_broadcast(
                    [128, 2, n_cols]))
            for dt in range(DT):
                eng = nc.vector if dt % 2 == 0 else nc.gpsimd
                t1 = sb.tile([128, n_cols], f32, tag=f"{tag}_t1")
                eng.tensor_tensor(t1[:], sum_f32[:, dt], bcst[:, 0], op=Alu.subtract)
                eng.tensor_tensor(t1[:], t1[:], bcst[:, 1], op=Alu.mult)
                if out_f32 is not None:
                    eng.tensor_scalar(out_f32[:, dt], t1[:], g_col[:, dt:dt + 1],
                                      b_col[:, dt:dt + 1], Alu.mult, Alu.add)
                    nc.scalar.copy(out_bf[:, dt], out_f32[:, dt])
                else:
                    eng.tensor_scalar(out_bf[:, dt], t1[:], g_col[:, dt:dt + 1],
                                      b_col[:, dt:dt + 1], Alu.mult, Alu.add)
            return bcst

        # ==================================== layer-1 global partials + AR #1
        ar1_in = dram.tile([65, 12], f32, tag="ar1_in")
        ar1_out = dram.tile([65, 12], f32, tag="ar1_out")
        with (
            tc.tile_pool(name="g1_sb", bufs=2) as gsb,
            tc.tile_pool(name="g1_w", bufs=1) as gw,
        ):
            wqgt = load_w(gw, "wqg0", DT, D)
            wkgt = load_w(gw, "wkg0", DT, D)
            wvgt = load_w(gw, "wvg0", DT, D)
            bkg_col = bias_col("bkg0")
            bvg_bc = bias_bcast("bvg0", gsb)

            vg_bf = gsb.tile([128, 4, D], bf16, tag="vg1")
            kgT_bf = gsb.tile([128, DT, 512], bf16, tag="kg1")
            with tc.tile_pool(name="g1_ps", bufs=1, space="PSUM") as gps:
                # qg row = x0[CLS]^T Wqg + bqg
                qg_ps = row_proj(gps, wqgt, x0T_bf[:, :, 1024:1025].rearrange(
                    "p k one -> p (k one)"), DT, D, "qg1")
                qg_row = gsb.tile([1, D], f32, tag="qg1_row")
                bqg_row = gsb.tile([1, D], f32, tag="bqg_row")
                nc.sync.dma_start(bqg_row[:], wname["bqg0"][:][None])
                nc.vector.tensor_tensor(qg_row[:], qg_ps[:], bqg_row[:], op=Alu.add)
                qg_col = col_bounce(qg_row[:], DT, "qg1", bf16)

                nat_proj(gps, gsb, wvgt, x0T_bf, [256 + 128 * t for t in range(4)],
                         bvg_bc, vg_bf, "vg1")
                xT_proj(gps, gsb, wkgt, x0T_bf, 256, 768, bkg_col, kgT_bf, "kg1")

            part_sb = gsb.tile([65, 12], f32, tag="part1")
            expg = gsb.tile([128, 12, 4], bf16, tag="expg1")
            with tc.tile_pool(name="g1_ps2", bufs=1, space="PSUM") as gps2:
                for h in range(H):
                    po, pk = (h % 2) * 64, h // 2
                    for t in range(4):
                        sg = gps2.tile([128, 1], f32, tag="sg1", space="PSUM", bufs=2)
                        nc.tensor.matmul(sg[:, :], kgT_bf[po:po + 64, pk, 128 * t:128 * (t + 1)],
                                         qg_col[po:po + 64, pk:pk + 1], start=True, stop=True)
                        nc.scalar.activation(expg[:, h, t:t + 1], sg[:, :], Act.Exp,
                                             scale=SCALE)
                    num = gps2.tile([64, 1], f32, tag="num1", space="PSUM", bufs=2)
                    for t in range(4):
                        nc.tensor.matmul(num[:, :], vg_bf[:, t, h * 64:(h + 1) * 64],
                                         expg[:, h, t:t + 1], start=(t == 0), stop=(t == 3))
                    nc.vector.tensor_copy(part_sb[0:64, h:h + 1], num[:, :])
                    den = gps2.tile([1, 4], f32, tag="den1", space="PSUM", bufs=2)
                    nc.tensor.matmul(den[:, :], ones_bf[:], expg[:, h, 0:4],
                                     start=True, stop=True)
                    nc.vector.reduce_sum(part_sb[64:65, h:h + 1], den[:, :],
                                         axis=mybir.AxisListType.X)
            nc.sync.dma_start(ar1_in[:], part_sb[:])
            nc.gpsimd.collective_compute(
                "AllReduce", Alu.add, replica_groups=RG,
                ins=[ar1_in.opt()], outs=[ar1_out.opt()])

        # ============================================= layer-1 Q/K/V + window
        cm_qkv = tc.tile_pool(name="P_qkv", bufs=1)
        pqkv = cm_qkv.__enter__()
        QT_bf = pqkv.tile([128, DT, 512], bf16, tag="QT_bf")
        KT_bf = pqkv.tile([128, DT, EXT], bf16, tag="KT_bf")
        V_bf = pqkv.tile([128, 8, D], bf16, tag="V_bf")
        vcls_bf = pqkv.tile([1, D], bf16, tag="vcls_bf")
        with (
            tc.tile_pool(name="qkv_sb", bufs=2) as qsb,
            tc.tile_pool(name="qkv_w", bufs=1) as qw,
        ):
            wqt = load_w(qw, "wq0", DT, D)
            wkt = load_w(qw, "wk0", DT, D)
            wvt = load_w(qw, "wv0", DT, D)
            bq_col = bias_col("bq0")
            bk_col = bias_col("bk0")
            bv_bc = bias_bcast("bv0", qsb)
            with tc.tile_pool(name="qkv_ps", bufs=1, space="PSUM") as qps:
                xT_proj(qps, qsb, wqt, x0T_bf, 256, 768, bq_col, QT_bf, "QT")
                # KT over all 1025 ext cols
                for ot in range(DT):
                    for s0, s1 in ((0, 512), (512, 1024), (1024, 1025)):
                        acc = qps.tile([128, 512], f32, tag="KT_ps", space="PSUM", bufs=2)
                        for kt in range(DT):
                            nc.tensor.matmul(acc[:, : s1 - s0],
                                             wkt[:, kt, ot * 128:(ot + 1) * 128],
                                             x0T_bf[:, kt, s0:s1],
                                             start=(kt == 0), stop=(kt == DT - 1))
                        nc.vector.tensor_scalar(KT_bf[:, ot, s0:s1], acc[:, : s1 - s0],
                                                bk_col[:, ot:ot + 1], None, Alu.add)
                nat_proj(qps, qsb, wvt, x0T_bf, [128 * t for t in range(8)], bv_bc,
                         V_bf, "V")
            with tc.tile_pool(name="vcls_ps", bufs=1, space="PSUM") as vps:
                vc = row_proj(vps, wvt, x0T_bf[:, :, 1024:1025].rearrange(
                    "p k one -> p (k one)"), DT, D, "vcls")
                nc.vector.tensor_tensor(vcls_bf[:], vc[:], bv_bc[0:1], op=Alu.add)

        # window attention -> attnT numerators + L denominators
        cm_att = tc.tile_pool(name="P_att", bufs=1)
        patt = cm_att.__enter__()
        attnT = patt.tile([128, DT, 513], bf16, tag="attnT")
        Lden = patt.tile([1, 12, 513], f32, tag="Lden")
        masks_sb = patt.tile([128, 2, 6, 256], bf16, tag="masks_sb")
        nc.sync.dma_start(
            masks_sb[:], masks_d[:].rearrange("c (k p) q -> p c k q", p=128))
        range_sb = patt.tile([128, 2, 2], f32, tag="range_sb")
        nc.sync.dma_start(range_sb[:], range_d[:].rearrange("c k p -> p c k"))
        Lr = patt.tile([1, 12, 512], f32, tag="Lr")
        with (
            tc.tile_pool(name="att_sb", bufs=3) as asb,
            tc.tile_pool(name="att_ps", bufs=2, space="PSUM") as aps,
        ):
            for cc in range(2):
                for h in range(H):
                    po, pk = (h % 2) * 64, h // 2
                    expT = asb.tile([128, 6, 256], bf16, tag="expT")
                    for kb in range(6):
                        sc = aps.tile([128, 256], f32, tag="sc", space="PSUM", bufs=2)
                        nc.tensor.matmul(
                            sc[:],
                            KT_bf[po:po + 64, pk, 256 * cc + 128 * kb: 256 * cc + 128 * (kb + 1)],
                            QT_bf[po:po + 64, pk, 256 * cc: 256 * (cc + 1)],
                            start=True, stop=True)
                        if kb in (2, 3):
                            nc.scalar.activation(expT[:, kb], sc[:], Act.Exp,
                                                 scale=SCALE,
                                                 bias=range_sb[:, cc, kb - 2: kb - 1])
                        else:
                            nc.vector.tensor_tensor(sc[:], sc[:], masks_sb[:, cc, kb],
                                                    op=Alu.add)
                            nc.scalar.activation(expT[:, kb], sc[:], Act.Exp, scale=SCALE)
                    s0p = aps.tile([1, 256], f32, tag="s0", space="PSUM", bufs=2)
                    nc.tensor.matmul(s0p[:], KT_bf[po:po + 64, pk, 1024:1025],
                                     QT_bf[po:po + 64, pk, 256 * cc:256 * (cc + 1)],
                                     start=True, stop=True)
                    e0 = asb.tile([1, 256], bf16, tag="e0")
                    nc.scalar.activation(e0[:], s0p[:], Act.Exp, scale=SCALE)
                    # denominator
                    dn = aps.tile([1, 256], f32, tag="dn", space="PSUM", bufs=2)
                    for kb in range(6):
                        nc.tensor.matmul(dn[:], ones_bf[:], expT[:, kb],
                                         start=(kb == 0), stop=(kb == 5))
                    nc.vector.tensor_tensor(Lden[:, h, 256 * cc:256 * (cc + 1)],
                                            dn[:], e0[:], op=Alu.add)
                    nc.vector.reciprocal_approx_fast(
                        out=Lr[:, h, 256 * cc:256 * (cc + 1)],
                        in_=Lden[:, h, 256 * cc:256 * (cc + 1)])
                    # A @ V
                    av = aps.tile([64, 256], f32, tag="av", space="PSUM", bufs=2)
                    for kb in range(6):
                        nc.tensor.matmul(av[:], V_bf[:, 2 * cc + kb, h * 64:(h + 1) * 64],
                                         expT[:, kb], start=(kb == 0), stop=False)
                    nc.tensor.matmul(av[:], vcls_bf[:, h * 64:(h + 1) * 64], e0[:],
                                     start=False, stop=True)
                    nc.vector.tensor_copy(attnT[po:po + 64, pk, 256 * cc:256 * (cc + 1)],
                                          av[:])

        # og1 columns [128, DT]: num interleaved + den broadcast, then divide
        og1_col = persist.tile([128, DT], f32, tag="og1_col")
        og1_den = persist.tile([128, DT], f32, tag="og1_den")
        for h2 in range(2):
            nc.gpsimd.dma_start(
                og1_col[h2 * 64:(h2 + 1) * 64, :],
                ar1_out[0:64].rearrange("p (k h) -> h p k", h=2)[h2])
            nc.gpsimd.dma_start(
                og1_den[h2 * 64:(h2 + 1) * 64, :],
                ar1_out[64:65].rearrange("one (k h) -> h one k", h=2)[h2].to_broadcast(
                    [64, DT]))
        nc.vector.reciprocal_approx_fast(out=og1_den[:], in_=og1_den[:])
        nc.vector.tensor_tensor(og1_col[:], og1_col[:], og1_den[:], op=Alu.mult)
        nc.vector.tensor_copy(dbg_sb[:, 12:18], og1_col[:])

        # normalize + og column + owner blend -> attn_norm_bf
        cm_nrm = tc.tile_pool(name="P_nrm", bufs=1)
        pnrm = cm_nrm.__enter__()
        attn_nbf = pnrm.tile([128, DT, 513], bf16, tag="attn_nbf")
        with tc.tile_pool(name="nrm_sb", bufs=2) as nsb:
            lr_d = dram.tile([12, 512], f32, tag="lr_d")
            nc.sync.dma_start(lr_d[:][None], Lr[:])
            lr_bc = nsb.tile([128, DT, 512], f32, tag="lr_bc")
            for h2 in range(2):
                nc.sync.dma_start(
                    lr_bc[h2 * 64:(h2 + 1) * 64],
                    lr_d[:].rearrange("(k h) f -> h k f", h=2)[h2][None].to_broadcast(
                        [64, DT, 512]))
            for dt in range(DT):
                nc.vector.tensor_tensor(attn_nbf[:, dt, 0:512], attnT[:, dt, 0:512],
                                        lr_bc[:, dt], op=Alu.mult)
                nc.vector.tensor_copy(attn_nbf[:, dt, 512:513], og1_col[:, dt:dt + 1])
            # owner blend on query col 0 (global position 0)
            d0 = nsb.tile([128, DT, 1], f32, tag="d0")
            nc.vector.tensor_tensor(d0[:], og1_col[:, :, None], attn_nbf[:, :, 0:1],
                                    op=Alu.subtract)
            nc.vector.tensor_tensor(d0[:], d0[:], own_sb[:, 0:1, None].to_broadcast(
                [128, DT, 1]), op=Alu.mult)
            nc.vector.tensor_tensor(attn_nbf[:, :, 0:1], attn_nbf[:, :, 0:1], d0[:],
                                    op=Alu.add)
        nc.vector.tensor_copy(dbg_sb[:, 6:12], attn_nbf[:, :, 0])

        # =============================================== O proj + LN1 + FFN
        with (
            tc.tile_pool(name="o_sb", bufs=2) as osb,
            tc.tile_pool(name="o_w", bufs=1) as ow,
            tc.tile_pool(name="o_ps", bufs=2, space="PSUM") as ops_,
        ):
            wot = load_w(ow, "wo0", DT, D)
            bo_col = bias_col("bo0")
            sum1 = osb.tile([128, DT, 513], f32, tag="sum1", bufs=1)
            for ot in range(DT):
                for s0, s1 in ((0, 512), (512, 513)):
                    acc = ops_.tile([128, 512], f32, tag="o_ps", space="PSUM", bufs=2)
                    for kt in range(DT):
                        nc.tensor.matmul(acc[:, : s1 - s0],
                                         wot[:, kt, ot * 128:(ot + 1) * 128],
                                         attn_nbf[:, kt, s0:s1],
                                         start=(kt == 0), stop=(kt == DT - 1))
                    t = osb.tile([128, 513], f32, tag="o_t")
                    nc.vector.tensor_scalar(t[:, s0:s1], acc[:, : s1 - s0],
                                            bo_col[:, ot:ot + 1], None, Alu.add)
                    nc.vector.tensor_tensor(sum1[:, ot, s0:s1], t[:, s0:s1],
                                            x0T_f32[:, ot, s0:s1], op=Alu.add)
            g1c = bias_col("ln1_g0")
            b1c = bias_col("ln1_b0")
            ln_xT(ops_, osb, sum1, 513, g1c, b1c, xa_bf, xa_f32, "ln1")
        nc.vector.tensor_copy(dbg_sb[:, 18:24], xa_f32[:, :, 0])

        # close attention-lifetime pools before the FFN phase (LIFO)
        cm_nrm.__exit__(None, None, None)
        cm_att.__exit__(None, None, None)
        cm_qkv.__exit__(None, None, None)
        cm_x0.__exit__(None, None, None)
        with (
            tc.tile_pool(name="f_sb", bufs=2) as fsb,
            tc.tile_pool(name="f_w", bufs=1) as fw,
            tc.tile_pool(name="f_ps", bufs=2, space="PSUM") as fps,
        ):
            w1t = load_w(fw, "w10", DT, FF)
            b1_col = bias_col("b10", FT)
            hT_bf = fsb.tile([128, FT, 513], bf16, tag="hT_bf", bufs=1)
            for ft in range(FT):
                for s0, s1 in ((0, 512), (512, 513)):
                    acc = fps.tile([128, 512], f32, tag="h_ps", space="PSUM", bufs=2)
                    for kt in range(DT):
                        nc.tensor.matmul(acc[:, : s1 - s0],
                                         w1t[:, kt, ft * 128:(ft + 1) * 128],
                                         xa_bf[:, kt, s0:s1],
                                         start=(kt == 0), stop=(kt == DT - 1))
                    nc.scalar.activation(hT_bf[:, ft, s0:s1], acc[:, : s1 - s0],
                                         Act.Gelu, bias=b1_col[:, ft:ft + 1])
            w2t = load_w(fw, "w20", FT, D)
            b2_col = bias_col("b20")
            sum2 = fsb.tile([128, DT, 513], f32, tag="sum2", bufs=1)
            for ot in range(DT):
                for s0, s1 in ((0, 512), (512, 513)):
                    acc = fps.tile([128, 512], f32, tag="f2_ps", space="PSUM", bufs=2)
                    for kt in range(FT):
                        nc.tensor.matmul(acc[:, : s1 - s0],
                                         w2t[:, kt, ot * 128:(ot + 1) * 128],
                                         hT_bf[:, kt, s0:s1],
                                         start=(kt == 0), stop=(kt == FT - 1))
                    t = fsb.tile([128, 513], f32, tag="f2_t")
                    nc.vector.tensor_scalar(t[:, s0:s1], acc[:, : s1 - s0],
                                            b2_col[:, ot:ot + 1], None, Alu.add)
                    nc.vector.tensor_tensor(sum2[:, ot, s0:s1], t[:, s0:s1],
                                            xa_f32[:, ot, s0:s1], op=Alu.add)
            g2c = bias_col("ln2_g0")
            b2c = bias_col("ln2_b0")
            st_bc = ln_xT(fps, fsb, sum2, 513, g2c, b2c, x1T_bf, None, "ln2")
            # f32 CLS column of x1 for the tail residual
            t1c = fsb.tile([128, DT, 1], f32, tag="t1c")
            nc.vector.tensor_tensor(t1c[:], sum2[:, :, 512:513],
                                    st_bc[:, 0:1, 512:513].to_broadcast([128, DT, 1]),
                                    op=Alu.subtract)
            nc.vector.tensor_tensor(t1c[:], t1c[:],
                                    st_bc[:, 1:2, 512:513].to_broadcast([128, DT, 1]),
                                    op=Alu.mult)
            for dt in range(DT):
                nc.vector.tensor_scalar(x1cls_col[:, dt:dt + 1], t1c[:, dt],
                                        g2c[:, dt:dt + 1], b2c[:, dt:dt + 1],
                                        Alu.mult, Alu.add)
        nc.vector.tensor_copy(dbg_sb[:, 24:30], x1cls_col[:])

        # ==================================== layer-2 global partials + AR #2
        ar2_in = dram.tile([65, 12], f32, tag="ar2_in")
        ar2_out = dram.tile([65, 12], f32, tag="ar2_out")
        with (
            tc.tile_pool(name="g2_sb", bufs=2) as gsb,
            tc.tile_pool(name="g2_w", bufs=1) as gw,
        ):
            wqgt = load_w(gw, "wqg1", DT, D)
            wkgt = load_w(gw, "wkg1", DT, D)
            wvgt = load_w(gw, "wvg1", DT, D)
            bkg_col = bias_col("bkg1")
            bvg_bc = bias_bcast("bvg1", gsb)
            vg_bf = gsb.tile([128, 4, D], bf16, tag="vg2")
            kgT_bf = gsb.tile([128, DT, 512], bf16, tag="kg2")
            with tc.tile_pool(name="g2_ps", bufs=1, space="PSUM") as gps:
                qg_ps = row_proj(gps, wqgt, x1T_bf[:, :, 512:513].rearrange(
                    "p k one -> p (k one)"), DT, D, "qg2")
                qg_row = gsb.tile([1, D], f32, tag="qg2_row")
                bqg_row = gsb.tile([1, D], f32, tag="bqg2_row")
                nc.sync.dma_start(bqg_row[:], wname["bqg1"][:][None])
                nc.vector.tensor_tensor(qg_row[:], qg_ps[:], bqg_row[:], op=Alu.add)
                qg_col = col_bounce(qg_row[:], DT, "qg2", bf16)

                nat_proj(gps, gsb, wvgt, x1T_bf, [128 * t for t in range(4)],
                         bvg_bc, vg_bf, "vg2")
                xT_proj(gps, gsb, wkgt, x1T_bf, 0, 512, bkg_col, kgT_bf, "kg2")

            part_sb = gsb.tile([65, 12], f32, tag="part2")
            expg = gsb.tile([128, 12, 4], bf16, tag="expg2")
            with tc.tile_pool(name="g2_ps2", bufs=1, space="PSUM") as gps2:
                for h in range(H):
                    po, pk = (h % 2) * 64, h // 2
                    for t in range(4):
                        sg = gps2.tile([128, 1], f32, tag="sg2", space="PSUM", bufs=2)
                        nc.tensor.matmul(sg[:, :], kgT_bf[po:po + 64, pk, 128 * t:128 * (t + 1)],
                                         qg_col[po:po + 64, pk:pk + 1], start=True, stop=True)
                        nc.scalar.activation(expg[:, h, t:t + 1], sg[:, :], Act.Exp,
                                             scale=SCALE)
                    num = gps2.tile([64, 1], f32, tag="num2", space="PSUM", bufs=2)
                    for t in range(4):
                        nc.tensor.matmul(num[:, :], vg_bf[:, t, h * 64:(h + 1) * 64],
                                         expg[:, h, t:t + 1], start=(t == 0), stop=(t == 3))
                    nc.vector.tensor_copy(part_sb[0:64, h:h + 1], num[:, :])
                    den = gps2.tile([1, 4], f32, tag="den2", space="PSUM", bufs=2)
                    nc.tensor.matmul(den[:, :], ones_bf[:], expg[:, h, 0:4],
                                     start=True, stop=True)
                    nc.vector.reduce_sum(part_sb[64:65, h:h + 1], den[:, :],
                                         axis=mybir.AxisListType.X)
            nc.sync.dma_start(ar2_in[:], part_sb[:])
            nc.gpsimd.collective_compute(
                "AllReduce", Alu.add, replica_groups=RG,
                ins=[ar2_in.opt()], outs=[ar2_out.opt()])

        # ============================================================== tail
        with (
            tc.tile_pool(name="t_sb", bufs=1) as tsb,
            tc.tile_pool(name="t_w", bufs=1) as tw,
        ):
            og2_col = tsb.tile([128, DT], f32, tag="og2_col")
            og2_den = tsb.tile([128, DT], f32, tag="og2_den")
            for h2 in range(2):
                nc.gpsimd.dma_start(
                    og2_col[h2 * 64:(h2 + 1) * 64, :],
                    ar2_out[0:64].rearrange("p (k h) -> h p k", h=2)[h2])
                nc.gpsimd.dma_start(
                    og2_den[h2 * 64:(h2 + 1) * 64, :],
                    ar2_out[64:65].rearrange("one (k h) -> h one k", h=2)[h2]
                    .to_broadcast([64, DT]))
            nc.vector.reciprocal_approx_fast(out=og2_den[:], in_=og2_den[:])
            nc.vector.tensor_tensor(og2_col[:], og2_col[:], og2_den[:], op=Alu.mult)
            nc.vector.tensor_copy(dbg_sb[:, 30:36], og2_col[:])
            og2_bf = tsb.tile([128, DT], bf16, tag="og2_bf")
            nc.vector.tensor_copy(og2_bf[:], og2_col[:])

            wot = load_w(tw, "wo1", DT, D)

            # x1cls row via bounce
            x1d = dram.tile([D], f32, tag="x1cls_d")
            nc.sync.dma_start(x1d[:].rearrange("(k p) -> p k", p=128), x1cls_col[:])
            x1row = tsb.tile([1, D], f32, tag="x1row")
            nc.sync.dma_start(x1row[:], x1d[:][None])

            def row_ln(src_ps_row, resid_row, gname, bname_ln, proj_bias, tag):
                """x = LN(resid + src + proj_bias_row) in row layout [1, 768]."""
                brow = tsb.tile([1, D], f32, tag=f"{tag}_brow")
                nc.sync.dma_start(brow[:], wname[proj_bias][:][None])
                srow = tsb.tile([1, D], f32, tag=f"{tag}_srow")
                nc.vector.tensor_tensor(srow[:], src_ps_row[:], brow[:], op=Alu.add)
                nc.vector.tensor_tensor(srow[:], srow[:], resid_row[:], op=Alu.add)
                stats = tsb.tile([1, 3, 6], f32, tag=f"{tag}_bn")
                for sg3 in range(3):
                    nc.vector.bn_stats(stats[:, sg3], srow[:, sg3 * 256:(sg3 + 1) * 256])
                mv = tsb.tile([1, 2], f32, tag=f"{tag}_mv")
                nc.vector.bn_aggr(mv[:], stats[:])
                sd = tsb.tile([1, 1], f32, tag=f"{tag}_sd")
                nc.scalar.activation(sd[:], mv[:, 1:2], Act.Sqrt, bias=eps1[:])
                nc.vector.reciprocal_approx_fast(out=sd[:], in_=sd[:])
                xr = tsb.tile([1, D], f32, tag=f"{tag}_x")
                nc.vector.tensor_scalar(xr[:], srow[:], mv[:, 0:1], sd[:, 0:1],
                                        Alu.subtract, Alu.mult)
                grow = tsb.tile([1, D], f32, tag=f"{tag}_grow")
                nc.sync.dma_start(grow[:], wname[gname][:][None])
                nc.vector.tensor_tensor(xr[:], xr[:], grow[:], op=Alu.mult)
                brow2 = tsb.tile([1, D], f32, tag=f"{tag}_brow2")
                nc.sync.dma_start(brow2[:], wname[bname_ln][:][None])
                nc.vector.tensor_tensor(xr[:], xr[:], brow2[:], op=Alu.add)
                return srow, xr

            with tc.tile_pool(name="tpsA", bufs=1, space="PSUM") as tpsA:
                a2 = row_proj(tpsA, wot, og2_bf, DT, D, "a2")
                _, x2a_row = row_ln(a2, x1row, "ln1_g1", "ln1_b1", "bo1", "tln1")
            x2a_col = col_bounce(x2a_row[:], DT, "x2a", bf16)

            with tc.tile_pool(name="t_w1", bufs=1) as tw1:
                w1t = load_w(tw1, "w11", DT, FF)
                with tc.tile_pool(name="tpsB", bufs=1, space="PSUM") as tpsB:
                    h2_ps = row_proj(tpsB, w1t, x2a_col, DT, FF, "h2")
                    b1row = tsb.tile([1, FF], f32, tag="b1row")
                    nc.sync.dma_start(b1row[:], wname["b11"][:][None])
                    h2row = tsb.tile([1, FF], f32, tag="h2row")
                    nc.vector.tensor_tensor(h2row[:], h2_ps[:], b1row[:], op=Alu.add)
                    nc.scalar.activation(h2row[:], h2row[:], Act.Gelu)
            h2_col = col_bounce(h2row[:], FT, "h2c", bf16)

            with tc.tile_pool(name="t_w2", bufs=1) as tw2:
                w2t = load_w(tw2, "w21", FT, D)
                with tc.tile_pool(name="tpsC", bufs=1, space="PSUM") as tpsC:
                    f2 = row_proj(tpsC, w2t, h2_col, FT, D, "f2")
                    _, x2_row = row_ln(f2, x2a_row, "ln2_g1", "ln2_b1", "b21", "tln2")

            x2_col = col_bounce(x2_row[:], DT, "x2c", f32)
            wc_sb = tsb.tile([128, DT, NL], f32, tag="wc_sb")
            nc.sync.dma_start(wc_sb[:], wc[:].rearrange("(k p) n -> p k n", p=128))
            with tc.tile_pool(name="tpsD", bufs=1, space="PSUM") as tpsD:
                lg = tpsD.tile([1, NL], f32, tag="lg_ps", space="PSUM")
                for kt in range(DT):
                    nc.tensor.matmul(lg[:], x2_col[:, kt:kt + 1], wc_sb[:, kt],
                                     start=(kt == 0), stop=(kt == DT - 1))
                bc_row = tsb.tile([1, NL], f32, tag="bc_row")
                nc.sync.dma_start(bc_row[:], bc[:][None])
                lrow = tsb.tile([1, NL], f32, tag="lrow")
                nc.vector.tensor_tensor(lrow[:], lg[:], bc_row[:], op=Alu.add)
            nc.sync.dma_start(logits_o[:], lrow[:])
            nc.sync.dma_start(dbg_o[:], dbg_sb[:])
        cm_xa.__exit__(None, None, None)

    nc.compile()
    return nc


# ------------------------------------------------------------------ host glue

def _prep_inputs(input_ids, attention_mask, params):
    p = {k: np.asarray(v) for k, v in params.items()}
    ids_np = np.asarray(input_ids).astype(np.int32)

    shared = {
        "tok_emb": _f32(p["tok_emb"]),
        "lne_g": _f32(p["ln_e_g"]), "lne_b": _f32(p["ln_e_b"]),
        "wc": _f32(p["Wc"]), "bc": _f32(p["bc"]),
    }
    for i in range(2):
        names = [("wqg", "Wqg"), ("wkg", "Wkg"), ("wvg", "Wvg"), ("wo", "Wo"),
                 ("w1", "W1"), ("w2", "W2")] + (
            [("wq", "Wq"), ("wk", "Wk"), ("wv", "Wv")] if i == 0 else [])
        for n, rn in names:
            shared[f"{n}{i}"] = _bf(p[rn][i])
        bn = [("bqg", "bqg"), ("bkg", "bkg"), ("bvg", "bvg"), ("bo", "bo"),
              ("b1", "b1"), ("b2", "b2"), ("ln1_g", "ln1_g"), ("ln1_b", "ln1_b"),
              ("ln2_g", "ln2_g"), ("ln2_b", "ln2_b")] + (
            [("bq", "bq"), ("bk", "bk"), ("bv", "bv")] if i == 0 else [])
        for n, rn in bn:
            shared[f"{n}{i}"] = _f32(p[rn][i])

    pos = _f32(p["pos_emb"])
    in_maps = []
    for c in range(8):
        b, j = c // 4, c % 4
        ext_idx, _ = _geometry(j)
        ids_ext = ids_np[b][ext_idx]
        ids_in = np.concatenate([ids_ext, ids_ext[-1:]]).reshape(1026, 1)
        m = dict(shared)
        m["ids"] = np.ascontiguousarray(ids_in)
        m["pos_ext"] = np.ascontiguousarray(pos[ext_idx])
        m["masks"] = _bf(_window_masks(j))
        m["range_bias"] = _range_bias(j)
        m["is_owner"] = np.full((128, 1), 1.0 if j == 0 else 0.0, np.float32)
        in_maps.append(m)
    return in_maps


def kernel(input_ids, attention_mask, params):
    from concourse.bass_utils import run_bass_kernel_spmd

    if "nc" not in _CACHE:
        _CACHE["nc"] = _build_program()
    nc = _CACHE["nc"]
    in_maps = _prep_inputs(input_ids, attention_mask, params)
    r = run_bass_kernel_spmd(nc, in_maps, core_ids=list(range(8)))
    _CACHE["last_results"] = r
    out = np.stack([r.results[0]["logits"][0], r.results[4]["logits"][0]], 0)
    return out.astype(np.float32)
